# revision 15
# baseline (speedup 1.0000x reference)
"""CKConv Trainium2 kernel.

Math (derived from the reference):
  out[b,o,l] = sum_i sum_{d=0}^{l} g[o,i,d] * x[b,i,l-d] + conv_bias[o]
  g[o,i,d]   = k_full[o,i,2047-d],  k_full = w3 @ h2 + b3
  h2 = sin(30*(w2 @ h1 + b2)), h1 = sin(30*(w1 @ t + b1)), t = linspace(-1,1,L)
  Feeding tr = -t (= reversed t) gives h2r/k_rev with k_rev[:, d] = k_full[:, 2047-d],
  so g[o,i,d] = k_rev[16*o+i, d].

Mapping (per core, data-parallel over batch b):
  - XS bank [128, 16*2560] fp16: XS[d'', 2560*i + c] = x[b,i, c-511-d''] (0 outside),
    built by one seed DMA of host-padded x + 7 log-doubling shift DMAs.
  - SIREN computed on device in fp16 matmuls (t and 30*w1 split hi/lo for accuracy)
    with fp32 range reduction (magic-number round) before the ACT Sin LUT
    (LUT domain is [-pi, pi]).
  - L3 produces K_revT[t][d'', 32*i+o] = k_rev[16*o+i, 128*t+d''] directly:
    16 matmuls lhsT=H2flat[:,128t:+128] ([33,128], ones row for b3), rhs=W3T [33,512].
  - Conv: 640 matmuls [K=128, M=32, N=512] fp16: for (p,t,i):
      psum[g] += K16[:, 512t+32i:+32].T @ XS[:, 2560i + 511 + 512p - 128t : +512]
    spread over 4 PE column groups (tile_position) with per-(p,g) psum accumulators.
  - Partials [128, 2048] fp32 -> HBM; host sums the 4 group partials + conv_bias.
"""
import numpy as np

OMEGA0 = 30.0
CIN, COUT, HID = 16, 32, 32
B, L = 8, 2048
PAD = 511          # left zero pad inside each XS row block
XSW = 2560         # per-i XS row width: PAD + L + 1
PI = float(np.pi)
TWO_PI = float(2 * np.pi)
MAGIC = 12582912.0  # 1.5 * 2**23, fp32 round-to-nearest trick
INV_2PI = float(1.0 / (2 * np.pi))

_COMPILED = {}
_KERNEL_OPTS = {"trace": False, "last_results": None}


def _split16(a):
    hi = a.astype(np.float16)
    lo = (a - hi.astype(np.float64)).astype(np.float16)
    return hi, lo


def _build_host_inputs(w1, b1, w2, b2, w3, b3):
    """Small host-side layout prep of the SIREN weights (fp64 for exactness)."""
    w1 = np.asarray(w1, np.float64)  # [32, 1]
    b1 = np.asarray(b1, np.float64)  # [32]
    w2 = np.asarray(w2, np.float64)  # [32, 32]
    b2 = np.asarray(b2, np.float64)  # [32]
    w3 = np.asarray(w3, np.float64)  # [512, 32]
    b3 = np.asarray(b3, np.float64)  # [512]

    t = np.linspace(-1.0, 1.0, L)
    tr = -t  # reversed t
    th, tl = _split16(tr)
    t4 = np.stack([th, tl, th, tl]).astype(np.float16)          # [4, L]

    w1s = OMEGA0 * w1[:, 0]                                      # [32]
    wh, wl = _split16(w1s)
    a1 = np.stack([wh, wh, wl, wl]).astype(np.float16)           # [4, 32]
    # pairing: (wh*th) + (wh*tl) + (wl*th) + (wl*tl) = w1s * tr (to ~2^-22)

    b1rep = np.tile((OMEGA0 * b1).astype(np.float32), 4)[:, None]   # [128,1]
    a2 = np.tile((OMEGA0 * w2.T).astype(np.float16), (4, 1))     # [128, 32]
    b2rep = np.tile((OMEGA0 * b2).astype(np.float32), 4)[:, None]   # [128,1]

    # W3T[c, 32*i+o] = w3[16*o+i, c]; row 32 = b3[16*o+i]
    w3t = np.zeros((33, 512), np.float16)
    oi = np.arange(512)
    o, i = oi // CIN, oi % CIN
    f = 32 * i + o
    w3t[:32, f] = w3[oi, :].T.astype(np.float16)
    w3t[32, f] = b3[oi].astype(np.float16)
    ones_row = np.ones((1, L), np.float16)
    return dict(t4=t4, a1=a1, b1rep=b1rep, a2=a2, b2rep=b2rep, w3t=w3t,
                ones_row=ones_row)


def _conv_tasks():
    """(p, t, i) task list and its round-robin split over 4 PE col groups."""
    tasks = []
    for p in range(4):
        for t in range(4 * p + 4):
            for i in range(CIN):
                tasks.append((p, t, i))
    groups = [[], [], [], []]
    for k, task in enumerate(tasks):
        groups[k % 4].append(task)
    return groups


def _gen():
    import concourse.bass as bass
    import concourse.mybir as mybir
    import concourse.tile as tile
    from concourse import bacc

    F32 = mybir.dt.float32
    F16 = mybir.dt.float16
    AF = mybir.ActivationFunctionType
    OP = mybir.AluOpType

    nc = bacc.Bacc()
    xpad = nc.dram_tensor("xpad", [CIN, XSW], F16, kind="ExternalInput")
    t4 = nc.dram_tensor("t4", [4, L], F16, kind="ExternalInput")
    a1 = nc.dram_tensor("a1", [4, 32], F16, kind="ExternalInput")
    b1rep = nc.dram_tensor("b1rep", [128, 1], F32, kind="ExternalInput")
    a2 = nc.dram_tensor("a2", [128, 32], F16, kind="ExternalInput")
    b2rep = nc.dram_tensor("b2rep", [128, 1], F32, kind="ExternalInput")
    w3t = nc.dram_tensor("w3t", [33, 512], F16, kind="ExternalInput")
    ones_row = nc.dram_tensor("ones_row", [1, L], F16, kind="ExternalInput")
    cbias = nc.dram_tensor("cbias", [32, 1], F32, kind="ExternalInput")
    out_res = nc.dram_tensor("out_res", [32, L], F32, kind="ExternalOutput")

    groups = _conv_tasks()

    with tile.TileContext(nc) as tc:
        with tc.tile_pool(name="pool", bufs=1) as pool, \
             tc.tile_pool(name="pps", bufs=1, space="PSUM") as pps:

            # ---------- load small inputs ----------
            t4t = pool.tile([4, L], F16)
            nc.sync.dma_start(t4t[:], t4[:, :])
            a1t = pool.tile([4, 32], F16)
            nc.sync.dma_start(a1t[:], a1[:, :])
            b1t = pool.tile([128, 1], F32)
            nc.sync.dma_start(b1t[:], b1rep[:, :])
            a2t = pool.tile([128, 32], F16)
            nc.sync.dma_start(a2t[:], a2[:, :])
            b2t = pool.tile([128, 1], F32)
            nc.sync.dma_start(b2t[:], b2rep[:, :])
            w3tt = pool.tile([33, 512], F16)
            nc.sync.dma_start(w3tt[:], w3t[:, :])
            cbt = pool.tile([32, 1], F32)
            nc.sync.dma_start(cbt[:], cbias[:, :])

            # ---------- XS bank build: 4 chains of 4 i's each ----------
            NG = 4       # i's per group
            GW = NG * XSW
            xss = [pool.tile([128, GW], F16, name=f"xs_{gg}", tag=f"xs{gg}")
                   for gg in range(4)]
            for gg in range(4):
                xs3 = xss[gg].rearrange("p (i c) -> p i c", i=NG)
                nc.sync.dma_start(xs3[0:1, :, :], xpad[NG * gg:NG * gg + NG, :])
                nc.vector.memset(xs3[:, :, 0:128], 0.0)
                for k in range(7):
                    n = 1 << k
                    nc.sync.dma_start(xs3[n:2 * n, :, n:XSW],
                                      xs3[0:n, :, 0:XSW - n])

            # ---------- SIREN L1 (stacked [128,512]) ----------
            ps1 = pps.tile([128, 512], F32)
            for a in range(4):
                nc.tensor.matmul(ps1[32 * a:32 * a + 32, :],
                                 a1t[:, :],
                                 t4t[:, 512 * a:512 * a + 512],
                                 start=True, stop=True,
                                 tile_position=(0, 32 * a))
            w_t = pool.tile([128, 512], F32)
            nc.vector.tensor_scalar(w_t[:], ps1[:], b1t[:], INV_2PI,
                                    OP.add, OP.mult)
            u_t = pool.tile([128, 512], F32)
            nc.vector.tensor_scalar(u_t[:], w_t[:], MAGIC, None, OP.add)
            n_t = pool.tile([128, 512], F32)
            nc.vector.tensor_scalar(n_t[:], u_t[:], MAGIC, None, OP.subtract)
            d_t = pool.tile([128, 512], F32)
            nc.vector.tensor_tensor(d_t[:], w_t[:], n_t[:], OP.subtract)
            h1 = pool.tile([128, 512], F16)
            nc.scalar.activation(h1[:], d_t[:], AF.Sin, scale=TWO_PI)

            # ---------- SIREN L2 ----------
            ps2 = pps.tile([128, 512], F32)
            for a in range(4):
                nc.tensor.matmul(ps2[32 * a:32 * a + 32, :],
                                 a2t[32 * a:32 * a + 32, :],
                                 h1[32 * a:32 * a + 32, :],
                                 start=True, stop=True,
                                 tile_position=(32 * a, 32 * a))
            w2_t = pool.tile([128, 512], F32)
            nc.vector.tensor_scalar(w2_t[:], ps2[:], b2t[:], INV_2PI,
                                    OP.add, OP.mult)
            u2_t = pool.tile([128, 512], F32)
            nc.vector.tensor_scalar(u2_t[:], w2_t[:], MAGIC, None, OP.add)
            n2_t = pool.tile([128, 512], F32)
            nc.vector.tensor_scalar(n2_t[:], u2_t[:], MAGIC, None, OP.subtract)
            d2_t = pool.tile([128, 512], F32)
            nc.vector.tensor_tensor(d2_t[:], w2_t[:], n2_t[:], OP.subtract)
            # H2 flat [33, 2048]: rows 0-31 features, row 32 ones
            h2 = pool.tile([33, L], F16)
            nc.sync.dma_start(h2[32:33, :], ones_row[:, :])
            for a in range(4):
                nc.scalar.activation(h2[0:32, 512 * a:512 * a + 512],
                                     d2_t[32 * a:32 * a + 32, :],
                                     AF.Sin, scale=TWO_PI)

            # ---------- SIREN L3 + Conv, interleaved t-major ----------
            k16 = pool.tile([128, 16 * 512], F16)
            accs = []
            for p in range(4):
                acc = pps.tile([128, 512], F32, name=f"acc_{p}", tag=f"acc{p}")
                accs.append(acc)
            # per-group task streams and start/stop bookkeeping
            started = set()
            last_touch = {}   # (p, g) -> emission index of last task
            seq = []          # emitted (g, p, t, i) in order
            nslots = max(len(gr) for gr in groups)
            for slot in range(nslots):
                for g in range(4):
                    if slot < len(groups[g]):
                        p, t, i = groups[g][slot]
                        seq.append((g, p, t, i))
                        last_touch[(p, g)] = len(seq) - 1
            for th in range(16):
                ps3 = pps.tile([128, 512], F32, name=f"ps3_{th}", tag="ps3", bufs=2)
                nc.tensor.matmul(ps3[:, :],
                                 h2[:, 128 * th:128 * th + 128],
                                 w3tt[:, :],
                                 start=True, stop=True)
                nc.vector.tensor_copy(k16[:, 512 * th:512 * th + 512], ps3[:, :])
            for idx, (g, p, t, i) in enumerate(seq):
                first = (p, g) not in started
                started.add((p, g))
                last = last_touch[(p, g)] == idx
                xs_g = xss[i // 4]
                col = XSW * (i % 4) + PAD + 512 * p - 128 * t
                nc.tensor.matmul(
                    accs[p][32 * g:32 * g + 32, :],
                    k16[:, 512 * t + 32 * i: 512 * t + 32 * i + 32],
                    xs_g[:, col:col + 512],
                    start=first, stop=last,
                    tile_position=(0, 32 * g))

            # ---------- reduce col groups + bias, write out ----------
            for p in range(4):
                sb = pool.tile([32, 512], F32, name=f"sb_{p}", tag="sbout", bufs=2)
                nc.vector.tensor_scalar(sb[:], accs[p][0:32, :], cbt[:], None,
                                        OP.add)
                for g in range(1, 4):
                    nc.vector.tensor_tensor(sb[:], sb[:],
                                            accs[p][32 * g:32 * g + 32, :],
                                            OP.add)
                nc.sync.dma_start(out_res[:, 512 * p:512 * p + 512], sb[:])

    nc.finalize()
    return nc


def _get_runner():
    """Build (once) a cached jitted shard_map runner for the 8-core SPMD kernel."""
    if "runner" in _COMPILED:
        return _COMPILED["runner"]

    import jax
    import numpy as np_
    from jax.sharding import Mesh, PartitionSpec
    from jax.experimental.shard_map import shard_map
    import concourse.mybir as mybir
    from concourse import bass2jax
    from concourse.bass2jax import _bass_exec_p, install_neuronx_cc_hook

    if "nc" not in _COMPILED:
        _COMPILED["nc"] = _gen()
    nc = _COMPILED["nc"]

    install_neuronx_cc_hook()

    partition_name = nc.partition_id_tensor.name if nc.partition_id_tensor else None
    in_names, out_names, out_avals, zero_outs = [], [], [], []
    for alloc in nc.m.functions[0].allocations:
        if not isinstance(alloc, mybir.MemoryLocationSet):
            continue
        name = alloc.memorylocations[0].name
        if alloc.kind == "ExternalInput":
            if name != partition_name:
                in_names.append(name)
        elif alloc.kind == "ExternalOutput":
            out_names.append(name)
            shape = tuple(alloc.tensor_shape)
            dtype = mybir.dt.np(alloc.dtype)
            out_avals.append(jax.core.ShapedArray(shape, dtype))
            zero_outs.append(np.zeros(shape, dtype))
    n_params = len(in_names)
    n_outs = len(out_avals)
    all_in_names = list(in_names) + list(out_names)
    if partition_name is not None:
        all_in_names.append(partition_name)
    donate = tuple(range(n_params, n_params + n_outs))

    def _body(*args):
        operands = list(args)
        if partition_name is not None:
            operands.append(bass2jax.partition_id_tensor())
        outs = _bass_exec_p.bind(
            *operands,
            out_avals=tuple(out_avals),
            in_names=tuple(all_in_names),
            out_names=tuple(out_names),
            lowering_input_output_aliases=(),
            sim_require_finite=True,
            sim_require_nnan=True,
            nc=nc,
        )
        return tuple(outs)

    devices = jax.devices()[:B]
    mesh = Mesh(np.asarray(devices, dtype=object), ("core",))
    in_specs = (PartitionSpec("core"),) * (n_params + n_outs)
    out_specs = (PartitionSpec("core"),) * len(out_names)
    sharded = jax.jit(
        shard_map(_body, mesh=mesh, in_specs=in_specs, out_specs=out_specs,
                  check_rep=False),
        donate_argnums=donate, keep_unused=True,
    )

    runner = dict(sharded=sharded, in_names=in_names, out_names=out_names,
                  out_avals=out_avals, zero_outs=zero_outs)
    _COMPILED["runner"] = runner
    return runner


def _run_spmd(in_maps):
    import numpy as np_
    r = _get_runner()
    n_cores = len(in_maps)
    per_core = [[np.asarray(m[name]) for name in r["in_names"]] for m in in_maps]
    concat_in = [np.concatenate([per_core[c][i] for c in range(n_cores)], axis=0)
                 for i in range(len(r["in_names"]))]
    concat_zeros = [np.zeros((n_cores * z.shape[0], *z.shape[1:]), z.dtype)
                    for z in r["zero_outs"]]
    out_arrs = r["sharded"](*concat_in, *concat_zeros)
    out_arrs = [np.asarray(a) for a in out_arrs]
    return [
        {name: out_arrs[i].reshape(n_cores, *r["out_avals"][i].shape)[c]
         for i, name in enumerate(r["out_names"])}
        for c in range(n_cores)
    ]


def _make_in_maps(x, conv_bias, host):
    cb = np.asarray(conv_bias, np.float32).reshape(32, 1)
    in_maps = []
    for b in range(B):
        xpad = np.zeros((CIN, XSW), np.float16)
        xpad[:, PAD:PAD + L] = x[b].astype(np.float16)
        in_maps.append(dict(xpad=xpad, cbias=cb, **host))
    return in_maps


def _postprocess(results):
    out = np.zeros((B, COUT, L), np.float32)
    for b in range(B):
        out[b] = results[b]["out_res"]
    return out


def kernel(x, w1, b1, w2, b2, w3, b3, conv_bias):
    x = np.asarray(x)
    host = _build_host_inputs(w1, b1, w2, b2, w3, b3)
    in_maps = _make_in_maps(x, conv_bias, host)
    results = _run_spmd(in_maps)
    return _postprocess(results)
